# revision 39
# baseline (speedup 1.0000x reference)
"""Trainium2 Bass kernel for nn_AttentionConvHead (windowed per-channel attention).

Math (per batch b, all channels d independent):
    Q = Wq @ q + bq ; K = Wk @ k + bk ; V = Wv @ v + bv        (1x1 convs)
    out[d,t,n] = sum_i softmax_i(Q[d,t,n] * Kpad[d,t+i,n]) * Vpad[d,t+i,n]
with K/V zero-padded by 3 on the time axis (pad contributes exp(0)=1 to the
softmax denominator and 0 to the numerator).

Distribution: pure data-parallel, one batch element per NeuronCore (B=8).

Per-core layout: partitions p = c + 64*g pack (channel, n-half); n (207,
padded to 208) splits into two groups of 104. Free dim is (t outer, n_local
inner) so a time shift is a contiguous free-dim offset of i*104.
Projections: 128x128 block-diagonal bf16 matmuls. Window sums: fp32 PSUM
accumulation via bf16 identity matmuls. exp + all PSUM->SBUF evictions on
ScalarE; score/value products (bf16 2x) + reciprocal + final mul on VectorE.
Phase A (K/V proj) and phase B (attention) are emission-interleaved so the
per-engine instruction streams pipeline across phases.
"""

import numpy as np

B, C, T, N = 8, 64, 128, 207
D = 64
KS, PAD = 7, 3
NPAD, NG, P = 208, 104, 128
F = T * NG                 # 13312 free positions per partition
TP = T + 2 * PAD           # 134 padded time steps
FPAD = TP * NG             # 13936
MM = 512                   # psum bank = 512 fp32 matmul columns
ACH = 2048                 # phase-A step size
A_STEPS = [(j, min(ACH, F - j)) for j in range(0, F, ACH)]
CHUNKS = [1536] * 8 + [1024]   # phase-B chunks (sum = F)

_CACHE = {}


def _build():
    import concourse.bacc as bacc
    import concourse.bass as bass
    import concourse.mybir as mybir
    from concourse.tile import TileContext

    f32 = mybir.dt.float32
    bf16 = mybir.dt.bfloat16
    AF = mybir.ActivationFunctionType

    nc = bacc.Bacc("TRN2", target_bir_lowering=False)

    xq = nc.declare_dram_parameter("xq", [P, F], bf16, isOutput=False)
    xk = nc.declare_dram_parameter("xk", [P, F], bf16, isOutput=False)
    xv = nc.declare_dram_parameter("xv", [P, F], bf16, isOutput=False)
    # wts: [wq | wk | wv | ident] as block-diag lhsT matrices, side by side
    wts = nc.declare_dram_parameter("wts", [P, 4 * P], bf16, isOutput=False)
    # bia: [bq | bk | bv] per-partition biases
    bia = nc.declare_dram_parameter("bia", [P, 3], f32, isOutput=False)
    out_d = nc.declare_dram_parameter("out", [P, F], bf16, isOutput=True)

    from contextlib import ExitStack

    with TileContext(nc) as tc, ExitStack() as ctx:
        consts = ctx.enter_context(tc.tile_pool(name="consts", bufs=1))
        xin = ctx.enter_context(tc.tile_pool(name="xin", bufs=4))
        big = ctx.enter_context(tc.tile_pool(name="big", bufs=1))
        work = ctx.enter_context(tc.tile_pool(name="work", bufs=4))
        outp = ctx.enter_context(tc.tile_pool(name="outp", bufs=2))
        psA = ctx.enter_context(tc.tile_pool(name="psA", bufs=2, space="PSUM"))
        psB = ctx.enter_context(tc.tile_pool(name="psB", bufs=1, space="PSUM"))

        wts_s = consts.tile([P, 4 * P], bf16, tag="wts")
        bia_s = consts.tile([P, 3], f32, tag="bia")
        nc.sync.dma_start(out=wts_s, in_=wts.ap())
        nc.sync.dma_start(out=bia_s, in_=bia.ap())
        wq_s = wts_s[:, 0:P]
        wk_s = wts_s[:, P : 2 * P]
        wv_s = wts_s[:, 2 * P : 3 * P]
        id_s = wts_s[:, 3 * P : 4 * P]
        bq_s = bia_s[:, 0:1]
        bk_s = bia_s[:, 1:2]
        bv_s = bia_s[:, 2:3]

        Kp = big.tile([P, FPAD], bf16, tag="Kp")
        Vp = big.tile([P, FPAD], bf16, tag="Vp")

        nc.vector.memset(Kp[:, 0 : PAD * NG], 0.0)
        nc.vector.memset(Kp[:, FPAD - PAD * NG : FPAD], 0.0)
        nc.vector.memset(Vp[:, 0 : PAD * NG], 0.0)
        nc.vector.memset(Vp[:, FPAD - PAD * NG : FPAD], 0.0)

        def emit_A(j0, ch):
            """DMA + project + evict one chunk of K and V (xq streamed along)."""
            kt = xin.tile([P, ACH], bf16, tag="xin")
            nc.sync.dma_start(out=kt[:, :ch], in_=xk.ap()[:, j0 : j0 + ch])
            vt = xin.tile([P, ACH], bf16, tag="xin")
            nc.sync.dma_start(out=vt[:, :ch], in_=xv.ap()[:, j0 : j0 + ch])
            for m0 in range(0, ch, MM):
                ps = psA.tile([P, MM], f32, tag="psA")
                nc.tensor.matmul(ps, wk_s, kt[:, m0 : m0 + MM], start=True, stop=True)
                nc.scalar.activation(
                    Kp[:, PAD * NG + j0 + m0 : PAD * NG + j0 + m0 + MM],
                    ps,
                    AF.Identity,
                    bias=bk_s,
                    scale=1.0,
                )
            return vt

        def emit_A_V(j0, ch, vt):
            for m0 in range(0, ch, MM):
                ps = psA.tile([P, MM], f32, tag="psA")
                nc.tensor.matmul(ps, wv_s, vt[:, m0 : m0 + MM], start=True, stop=True)
                nc.scalar.activation(
                    Vp[:, PAD * NG + j0 + m0 : PAD * NG + j0 + m0 + MM],
                    ps,
                    AF.Identity,
                    bias=bv_s,
                    scale=1.0,
                )

        def emit_Q(base, fc):
            """Stream this chunk of xq in, project, evict (+bias) to bf16."""
            qx = work.tile([P, fc], bf16, tag="qx", bufs=3)
            nc.sync.dma_start(out=qx, in_=xq.ap()[:, base : base + fc])
            qb = work.tile([P, fc], bf16, tag="qb", bufs=3)
            for m0 in range(0, fc, MM):
                qp = psA.tile([P, MM], f32, tag="psA")
                nc.tensor.matmul(
                    qp, wq_s, qx[:, m0 : m0 + MM], start=True, stop=True
                )
                nc.scalar.activation(
                    qb[:, m0 : m0 + MM], qp, AF.Identity, bias=bq_s, scale=1.0
                )
            return qb

        def emit_SE(base, fc, qb):
            """Batched score mul (all 7 windows in one op) + in-place exp."""
            seb = work.tile([P, KS, 1536], bf16, tag="seb", bufs=2)
            qb_b = bass.AP(tensor=qb.tensor, offset=qb.offset, ap=[qb.ap[0], [0, KS], [1, fc]])
            kp_v = bass.AP(
                tensor=Kp.tensor, offset=Kp.offset + base, ap=[Kp.ap[0], [NG, KS], [1, fc]]
            )
            nc.vector.tensor_mul(seb[:, :, :fc], qb_b, kp_v)
            nc.scalar.activation(seb[:, 0:4, :fc], seb[:, 0:4, :fc], AF.Exp)
            nc.scalar.activation(seb[:, 4:KS, :fc], seb[:, 4:KS, :fc], AF.Exp)
            return seb

        def emit_B_tail(base, fc, seb):
            """Value products, PSUM window accumulation, normalize, DMA out."""
            den = psB.tile([P, 1536], f32, tag="den", bufs=1)
            num = psB.tile([P, 1536], f32, tag="num", bufs=1)
            wb7 = work.tile([P, KS, 1536], bf16, tag="wb7", bufs=2)
            vp_a = bass.AP(
                tensor=Vp.tensor, offset=Vp.offset + base, ap=[Vp.ap[0], [NG, 4], [1, fc]]
            )
            vp_b = bass.AP(
                tensor=Vp.tensor,
                offset=Vp.offset + base + 4 * NG,
                ap=[Vp.ap[0], [NG, KS - 4], [1, fc]],
            )
            nc.vector.tensor_mul(wb7[:, 0:4, :fc], seb[:, 0:4, :fc], vp_a)
            nc.vector.tensor_mul(wb7[:, 4:KS, :fc], seb[:, 4:KS, :fc], vp_b)
            for i in range(KS):
                eb = seb[:, i, :fc]
                wb = wb7[:, i, :fc]
                first, last = i == 0, i == KS - 1
                for m0 in range(0, fc, MM):
                    nc.tensor.matmul(
                        den[:, m0 : m0 + MM],
                        id_s,
                        eb[:, m0 : m0 + MM],
                        start=first,
                        stop=last,
                        skip_group_check=True,
                    )
                    nc.tensor.matmul(
                        num[:, m0 : m0 + MM],
                        id_s,
                        wb[:, m0 : m0 + MM],
                        start=first,
                        stop=last,
                        skip_group_check=True,
                    )
            r = work.tile([P, fc], f32, tag="r", bufs=2)
            nc.vector.reciprocal_approx_fast(out=r, in_=den[:, :fc])
            ot = outp.tile([P, fc], bf16, tag="ot")
            nc.vector.tensor_mul(ot, num[:, :fc], r)
            nc.sync.dma_start(out=out_d.ap()[:, base : base + fc], in_=ot)

        # Interleave phase A and B with a two-deep software pipeline skew:
        # at iteration c we emit Q-projection for chunk c, score-mul+exp for
        # chunk c-1, and the value/accumulate/normalize block for chunk c-2.
        # This keeps the qb-evict -> score-mul -> exp chain off the critical
        # path between consecutive (large) exp ops.
        b_chunks = []
        base = 0
        for fc in CHUNKS:
            b_chunks.append((base, fc))
            base += fc
        ai = 0
        qbs = {}
        sebs = {}
        pending_v = []
        nb = len(b_chunks)
        for c in range(nb + 2):
            if c < nb:
                base, fc = b_chunks[c]
                need = min(F, base + fc + PAD * NG)  # interior coverage needed
                while ai < len(A_STEPS) and A_STEPS[ai][0] < need:
                    j0, ch = A_STEPS[ai]
                    pending_v.append((j0, ch, emit_A(j0, ch)))
                    ai += 1
                qbs[c] = emit_Q(base, fc)
            if c >= 1 and c - 1 < nb:
                base, fc = b_chunks[c - 1]
                sebs[c - 1] = emit_SE(base, fc, qbs.pop(c - 1))
            while pending_v:
                emit_A_V(*pending_v.pop(0))
            if c >= 2:
                base, fc = b_chunks[c - 2]
                emit_B_tail(base, fc, sebs.pop(c - 2))

    nc.compile()
    return nc


def _pack(x, bf):
    """[C, T, N] f32 -> [128, F] bf16: partition p = c + 64g, free = t*104 + n_loc."""
    xp = np.zeros((C, T, NPAD), np.float32)
    xp[:, :, :N] = x
    return np.ascontiguousarray(
        xp.reshape(C, T, 2, NG).transpose(2, 0, 1, 3).reshape(P, F)
    ).astype(bf)


def _unpack(o):
    """[128, F] -> [D, T, N]."""
    o = np.asarray(o, np.float32)
    return np.ascontiguousarray(
        o.reshape(2, D, T, NG).transpose(1, 2, 0, 3).reshape(D, T, NPAD)[:, :, :N]
    )


def _lhsT_blockdiag(W):
    Z = np.zeros((P, P), np.float32)
    Z[:C, :D] = W.T
    Z[C:, D:] = W.T
    return Z


def _prep_in_maps(q, k, v, Wq, bq, Wk, bk, Wv, bv):
    import ml_dtypes

    bf = ml_dtypes.bfloat16
    wts = np.concatenate(
        [
            _lhsT_blockdiag(np.asarray(Wq, np.float32)),
            _lhsT_blockdiag(np.asarray(Wk, np.float32)),
            _lhsT_blockdiag(np.asarray(Wv, np.float32)),
            np.eye(P, dtype=np.float32),
        ],
        axis=1,
    ).astype(bf)
    bia = np.stack(
        [np.concatenate([b, b]) for b in (bq, bk, bv)], axis=1
    ).astype(np.float32)
    in_maps = []
    for b in range(B):
        in_maps.append(
            {
                "xq": _pack(np.asarray(q[b], np.float32), bf),
                "xk": _pack(np.asarray(k[b], np.float32), bf),
                "xv": _pack(np.asarray(v[b], np.float32), bf),
                "wts": wts,
                "bia": bia,
            }
        )
    return in_maps


def run(inputs, trace=False):
    """Build (cached), run on 8 cores, return (output, BassKernelResults)."""
    from concourse.bass_utils import run_bass_kernel_spmd

    if "nc" not in _CACHE:
        _CACHE["nc"] = _build()
    nc = _CACHE["nc"]
    in_maps = _prep_in_maps(**inputs)
    res = run_bass_kernel_spmd(nc, in_maps, core_ids=list(range(B)), trace=trace)
    out = np.stack([_unpack(np.asarray(res.results[b]["out"])) for b in range(B)])
    return out, res


def kernel(q, k, v, Wq, bq, Wk, bk, Wv, bv):
    out, _ = run(dict(q=q, k=k, v=v, Wq=Wq, bq=bq, Wk=Wk, bk=bk, Wv=Wv, bv=bv))
    return out


# revision 40
# speedup vs baseline: 1.0044x; 1.0044x over previous
"""Trainium2 Bass kernel for nn_AttentionConvHead (windowed per-channel attention).

Math (per batch b, all channels d independent):
    Q = Wq @ q + bq ; K = Wk @ k + bk ; V = Wv @ v + bv        (1x1 convs)
    out[d,t,n] = sum_i softmax_i(Q[d,t,n] * Kpad[d,t+i,n]) * Vpad[d,t+i,n]
with K/V zero-padded by 3 on the time axis (pad contributes exp(0)=1 to the
softmax denominator and 0 to the numerator).

Distribution: pure data-parallel, one batch element per NeuronCore (B=8).

Per-core layout: partitions p = c + 64*g pack (channel, n-half); n (207,
padded to 208) splits into two groups of 104. Free dim is (t outer, n_local
inner) so a time shift is a contiguous free-dim offset of i*104.
Projections: 128x128 block-diagonal bf16 matmuls. Window sums: fp32 PSUM
accumulation via bf16 identity matmuls. exp + all PSUM->SBUF evictions on
ScalarE; score/value products (bf16 2x) + reciprocal + final mul on VectorE.
Phase A (K/V proj) and phase B (attention) are emission-interleaved so the
per-engine instruction streams pipeline across phases.
"""

import numpy as np

B, C, T, N = 8, 64, 128, 207
D = 64
KS, PAD = 7, 3
NPAD, NG, P = 208, 104, 128
F = T * NG                 # 13312 free positions per partition
TP = T + 2 * PAD           # 134 padded time steps
FPAD = TP * NG             # 13936
MM = 512                   # psum bank = 512 fp32 matmul columns
ACH = 2048                 # phase-A step size
A_STEPS = [(j, min(ACH, F - j)) for j in range(0, F, ACH)]
CHUNKS = [1536] * 8 + [1024]   # phase-B chunks (sum = F)

_CACHE = {}


def _build():
    import concourse.bacc as bacc
    import concourse.bass as bass
    import concourse.mybir as mybir
    from concourse.tile import TileContext

    f32 = mybir.dt.float32
    bf16 = mybir.dt.bfloat16
    AF = mybir.ActivationFunctionType

    nc = bacc.Bacc("TRN2", target_bir_lowering=False)

    xq = nc.declare_dram_parameter("xq", [P, F], bf16, isOutput=False)
    xk = nc.declare_dram_parameter("xk", [P, F], bf16, isOutput=False)
    xv = nc.declare_dram_parameter("xv", [P, F], bf16, isOutput=False)
    # wts: [wq | wk | wv | ident] as block-diag lhsT matrices, side by side
    wts = nc.declare_dram_parameter("wts", [P, 4 * P], bf16, isOutput=False)
    # bia: [bq | bk | bv] per-partition biases
    bia = nc.declare_dram_parameter("bia", [P, 3], f32, isOutput=False)
    out_d = nc.declare_dram_parameter("out", [P, F], bf16, isOutput=True)

    from contextlib import ExitStack

    with TileContext(nc) as tc, ExitStack() as ctx:
        consts = ctx.enter_context(tc.tile_pool(name="consts", bufs=1))
        xin = ctx.enter_context(tc.tile_pool(name="xin", bufs=6))
        big = ctx.enter_context(tc.tile_pool(name="big", bufs=1))
        work = ctx.enter_context(tc.tile_pool(name="work", bufs=4))
        outp = ctx.enter_context(tc.tile_pool(name="outp", bufs=4))
        psA = ctx.enter_context(tc.tile_pool(name="psA", bufs=2, space="PSUM"))
        psB = ctx.enter_context(tc.tile_pool(name="psB", bufs=1, space="PSUM"))

        wts_s = consts.tile([P, 4 * P], bf16, tag="wts")
        bia_s = consts.tile([P, 3], f32, tag="bia")
        nc.sync.dma_start(out=wts_s, in_=wts.ap())
        nc.sync.dma_start(out=bia_s, in_=bia.ap())
        wq_s = wts_s[:, 0:P]
        wk_s = wts_s[:, P : 2 * P]
        wv_s = wts_s[:, 2 * P : 3 * P]
        id_s = wts_s[:, 3 * P : 4 * P]
        bq_s = bia_s[:, 0:1]
        bk_s = bia_s[:, 1:2]
        bv_s = bia_s[:, 2:3]

        Kp = big.tile([P, FPAD], bf16, tag="Kp")
        Vp = big.tile([P, FPAD], bf16, tag="Vp")

        nc.vector.memset(Kp[:, 0 : PAD * NG], 0.0)
        nc.vector.memset(Kp[:, FPAD - PAD * NG : FPAD], 0.0)
        nc.vector.memset(Vp[:, 0 : PAD * NG], 0.0)
        nc.vector.memset(Vp[:, FPAD - PAD * NG : FPAD], 0.0)

        def emit_A(j0, ch):
            """DMA + project + evict one chunk of K and V (xq streamed along)."""
            kt = xin.tile([P, ACH], bf16, tag="xin")
            nc.sync.dma_start(out=kt[:, :ch], in_=xk.ap()[:, j0 : j0 + ch])
            vt = xin.tile([P, ACH], bf16, tag="xin")
            nc.sync.dma_start(out=vt[:, :ch], in_=xv.ap()[:, j0 : j0 + ch])
            for m0 in range(0, ch, MM):
                ps = psA.tile([P, MM], f32, tag="psA")
                nc.tensor.matmul(ps, wk_s, kt[:, m0 : m0 + MM], start=True, stop=True)
                nc.scalar.activation(
                    Kp[:, PAD * NG + j0 + m0 : PAD * NG + j0 + m0 + MM],
                    ps,
                    AF.Identity,
                    bias=bk_s,
                    scale=1.0,
                )
            return vt

        def emit_A_V(j0, ch, vt):
            for m0 in range(0, ch, MM):
                ps = psA.tile([P, MM], f32, tag="psA")
                nc.tensor.matmul(ps, wv_s, vt[:, m0 : m0 + MM], start=True, stop=True)
                nc.scalar.activation(
                    Vp[:, PAD * NG + j0 + m0 : PAD * NG + j0 + m0 + MM],
                    ps,
                    AF.Identity,
                    bias=bv_s,
                    scale=1.0,
                )

        def emit_Q(base, fc):
            """Stream this chunk of xq in, project, evict (+bias) to bf16."""
            qx = work.tile([P, fc], bf16, tag="qx", bufs=3)
            nc.sync.dma_start(out=qx, in_=xq.ap()[:, base : base + fc])
            qb = work.tile([P, fc], bf16, tag="qb", bufs=3)
            for m0 in range(0, fc, MM):
                qp = psA.tile([P, MM], f32, tag="psA")
                nc.tensor.matmul(
                    qp, wq_s, qx[:, m0 : m0 + MM], start=True, stop=True
                )
                nc.scalar.activation(
                    qb[:, m0 : m0 + MM], qp, AF.Identity, bias=bq_s, scale=1.0
                )
            return qb

        def emit_SE(base, fc, qb):
            """Batched score mul (all 7 windows in one op) + in-place exp."""
            seb = work.tile([P, KS, 1536], bf16, tag="seb", bufs=2)
            qb_b = bass.AP(tensor=qb.tensor, offset=qb.offset, ap=[qb.ap[0], [0, KS], [1, fc]])
            kp_v = bass.AP(
                tensor=Kp.tensor, offset=Kp.offset + base, ap=[Kp.ap[0], [NG, KS], [1, fc]]
            )
            nc.vector.tensor_mul(seb[:, :, :fc], qb_b, kp_v)
            nc.scalar.activation(seb[:, 0:4, :fc], seb[:, 0:4, :fc], AF.Exp)
            nc.scalar.activation(seb[:, 4:KS, :fc], seb[:, 4:KS, :fc], AF.Exp)
            return seb

        def emit_B_tail(base, fc, seb):
            """Value products, PSUM window accumulation, normalize, DMA out."""
            den = psB.tile([P, 1536], f32, tag="den", bufs=1)
            num = psB.tile([P, 1536], f32, tag="num", bufs=1)
            wb7 = work.tile([P, KS, 1536], bf16, tag="wb7", bufs=2)
            vp_a = bass.AP(
                tensor=Vp.tensor, offset=Vp.offset + base, ap=[Vp.ap[0], [NG, 4], [1, fc]]
            )
            vp_b = bass.AP(
                tensor=Vp.tensor,
                offset=Vp.offset + base + 4 * NG,
                ap=[Vp.ap[0], [NG, KS - 4], [1, fc]],
            )
            nc.vector.tensor_mul(wb7[:, 0:4, :fc], seb[:, 0:4, :fc], vp_a)
            nc.vector.tensor_mul(wb7[:, 4:KS, :fc], seb[:, 4:KS, :fc], vp_b)
            for i in range(KS):
                eb = seb[:, i, :fc]
                wb = wb7[:, i, :fc]
                first, last = i == 0, i == KS - 1
                for m0 in range(0, fc, MM):
                    nc.tensor.matmul(
                        den[:, m0 : m0 + MM],
                        id_s,
                        eb[:, m0 : m0 + MM],
                        start=first,
                        stop=last,
                        skip_group_check=True,
                    )
                    nc.tensor.matmul(
                        num[:, m0 : m0 + MM],
                        id_s,
                        wb[:, m0 : m0 + MM],
                        start=first,
                        stop=last,
                        skip_group_check=True,
                    )
            r = work.tile([P, fc], f32, tag="r", bufs=2)
            nc.vector.reciprocal_approx_fast(out=r, in_=den[:, :fc])
            ot = outp.tile([P, fc], bf16, tag="ot")
            nc.vector.tensor_mul(ot, num[:, :fc], r)
            nc.sync.dma_start(out=out_d.ap()[:, base : base + fc], in_=ot)

        # Interleave phase A and B with a two-deep software pipeline skew:
        # at iteration c we emit Q-projection for chunk c, score-mul+exp for
        # chunk c-1, and the value/accumulate/normalize block for chunk c-2.
        # This keeps the qb-evict -> score-mul -> exp chain off the critical
        # path between consecutive (large) exp ops.
        b_chunks = []
        base = 0
        for fc in CHUNKS:
            b_chunks.append((base, fc))
            base += fc
        ai = 0
        qbs = {}
        sebs = {}
        pending_v = []
        nb = len(b_chunks)
        for c in range(nb + 2):
            if c < nb:
                base, fc = b_chunks[c]
                need = min(F, base + fc + PAD * NG)  # interior coverage needed
                while ai < len(A_STEPS) and A_STEPS[ai][0] < need:
                    j0, ch = A_STEPS[ai]
                    pending_v.append((j0, ch, emit_A(j0, ch)))
                    ai += 1
                qbs[c] = emit_Q(base, fc)
            if c >= 1 and c - 1 < nb:
                base, fc = b_chunks[c - 1]
                sebs[c - 1] = emit_SE(base, fc, qbs.pop(c - 1))
            while pending_v:
                emit_A_V(*pending_v.pop(0))
            if c >= 2:
                base, fc = b_chunks[c - 2]
                emit_B_tail(base, fc, sebs.pop(c - 2))

    nc.compile()
    return nc


def _pack(x, bf):
    """[C, T, N] f32 -> [128, F] bf16: partition p = c + 64g, free = t*104 + n_loc."""
    xp = np.zeros((C, T, NPAD), np.float32)
    xp[:, :, :N] = x
    return np.ascontiguousarray(
        xp.reshape(C, T, 2, NG).transpose(2, 0, 1, 3).reshape(P, F)
    ).astype(bf)


def _unpack(o):
    """[128, F] -> [D, T, N]."""
    o = np.asarray(o, np.float32)
    return np.ascontiguousarray(
        o.reshape(2, D, T, NG).transpose(1, 2, 0, 3).reshape(D, T, NPAD)[:, :, :N]
    )


def _lhsT_blockdiag(W):
    Z = np.zeros((P, P), np.float32)
    Z[:C, :D] = W.T
    Z[C:, D:] = W.T
    return Z


def _prep_in_maps(q, k, v, Wq, bq, Wk, bk, Wv, bv):
    import ml_dtypes

    bf = ml_dtypes.bfloat16
    wts = np.concatenate(
        [
            _lhsT_blockdiag(np.asarray(Wq, np.float32)),
            _lhsT_blockdiag(np.asarray(Wk, np.float32)),
            _lhsT_blockdiag(np.asarray(Wv, np.float32)),
            np.eye(P, dtype=np.float32),
        ],
        axis=1,
    ).astype(bf)
    bia = np.stack(
        [np.concatenate([b, b]) for b in (bq, bk, bv)], axis=1
    ).astype(np.float32)
    in_maps = []
    for b in range(B):
        in_maps.append(
            {
                "xq": _pack(np.asarray(q[b], np.float32), bf),
                "xk": _pack(np.asarray(k[b], np.float32), bf),
                "xv": _pack(np.asarray(v[b], np.float32), bf),
                "wts": wts,
                "bia": bia,
            }
        )
    return in_maps


def run(inputs, trace=False):
    """Build (cached), run on 8 cores, return (output, BassKernelResults)."""
    from concourse.bass_utils import run_bass_kernel_spmd

    if "nc" not in _CACHE:
        _CACHE["nc"] = _build()
    nc = _CACHE["nc"]
    in_maps = _prep_in_maps(**inputs)
    res = run_bass_kernel_spmd(nc, in_maps, core_ids=list(range(B)), trace=trace)
    out = np.stack([_unpack(np.asarray(res.results[b]["out"])) for b in range(B)])
    return out, res


def kernel(q, k, v, Wq, bq, Wk, bk, Wv, bv):
    out, _ = run(dict(q=q, k=k, v=v, Wq=Wq, bq=bq, Wk=Wk, bk=bk, Wv=Wv, bv=bv))
    return out


# revision 41
# speedup vs baseline: 1.0055x; 1.0012x over previous
"""Trainium2 Bass kernel for nn_AttentionConvHead (windowed per-channel attention).

Math (per batch b, all channels d independent):
    Q = Wq @ q + bq ; K = Wk @ k + bk ; V = Wv @ v + bv        (1x1 convs)
    out[d,t,n] = sum_i softmax_i(Q[d,t,n] * Kpad[d,t+i,n]) * Vpad[d,t+i,n]
with K/V zero-padded by 3 on the time axis (pad contributes exp(0)=1 to the
softmax denominator and 0 to the numerator).

Distribution: pure data-parallel, one batch element per NeuronCore (B=8).

Per-core layout: partitions p = c + 64*g pack (channel, n-half); n (207,
padded to 208) splits into two groups of 104. Free dim is (t outer, n_local
inner) so a time shift is a contiguous free-dim offset of i*104.
Projections: 128x128 block-diagonal bf16 matmuls. Window sums: fp32 PSUM
accumulation via bf16 identity matmuls. exp + all PSUM->SBUF evictions on
ScalarE; score/value products (bf16 2x) + reciprocal + final mul on VectorE.
Phase A (K/V proj) and phase B (attention) are emission-interleaved so the
per-engine instruction streams pipeline across phases.
"""

import numpy as np

B, C, T, N = 8, 64, 128, 207
D = 64
KS, PAD = 7, 3
NPAD, NG, P = 208, 104, 128
F = T * NG                 # 13312 free positions per partition
TP = T + 2 * PAD           # 134 padded time steps
FPAD = TP * NG             # 13936
MM = 512                   # psum bank = 512 fp32 matmul columns
ACH = 2048                 # phase-A step size
A_STEPS = [(j, min(ACH, F - j)) for j in range(0, F, ACH)]
CHUNKS = [1024] * 13   # phase-B chunks (sum = F)

_CACHE = {}


def _build():
    import concourse.bacc as bacc
    import concourse.bass as bass
    import concourse.mybir as mybir
    from concourse.tile import TileContext

    f32 = mybir.dt.float32
    bf16 = mybir.dt.bfloat16
    AF = mybir.ActivationFunctionType

    nc = bacc.Bacc("TRN2", target_bir_lowering=False)

    xq = nc.declare_dram_parameter("xq", [P, F], bf16, isOutput=False)
    xk = nc.declare_dram_parameter("xk", [P, F], bf16, isOutput=False)
    xv = nc.declare_dram_parameter("xv", [P, F], bf16, isOutput=False)
    # wts: [wq | wk | wv | ident] as block-diag lhsT matrices, side by side
    wts = nc.declare_dram_parameter("wts", [P, 4 * P], bf16, isOutput=False)
    # bia: [bq | bk | bv] per-partition biases
    bia = nc.declare_dram_parameter("bia", [P, 3], f32, isOutput=False)
    out_d = nc.declare_dram_parameter("out", [P, F], bf16, isOutput=True)

    from contextlib import ExitStack

    with TileContext(nc) as tc, ExitStack() as ctx:
        consts = ctx.enter_context(tc.tile_pool(name="consts", bufs=1))
        xin = ctx.enter_context(tc.tile_pool(name="xin", bufs=6))
        big = ctx.enter_context(tc.tile_pool(name="big", bufs=1))
        work = ctx.enter_context(tc.tile_pool(name="work", bufs=4))
        outp = ctx.enter_context(tc.tile_pool(name="outp", bufs=4))
        psA = ctx.enter_context(tc.tile_pool(name="psA", bufs=2, space="PSUM"))
        psB = ctx.enter_context(tc.tile_pool(name="psB", bufs=1, space="PSUM"))

        wts_s = consts.tile([P, 4 * P], bf16, tag="wts")
        bia_s = consts.tile([P, 3], f32, tag="bia")
        nc.sync.dma_start(out=wts_s, in_=wts.ap())
        nc.sync.dma_start(out=bia_s, in_=bia.ap())
        wq_s = wts_s[:, 0:P]
        wk_s = wts_s[:, P : 2 * P]
        wv_s = wts_s[:, 2 * P : 3 * P]
        id_s = wts_s[:, 3 * P : 4 * P]
        bq_s = bia_s[:, 0:1]
        bk_s = bia_s[:, 1:2]
        bv_s = bia_s[:, 2:3]

        Kp = big.tile([P, FPAD], bf16, tag="Kp")
        Vp = big.tile([P, FPAD], bf16, tag="Vp")

        nc.vector.memset(Kp[:, 0 : PAD * NG], 0.0)
        nc.vector.memset(Kp[:, FPAD - PAD * NG : FPAD], 0.0)
        nc.vector.memset(Vp[:, 0 : PAD * NG], 0.0)
        nc.vector.memset(Vp[:, FPAD - PAD * NG : FPAD], 0.0)

        def emit_A(j0, ch):
            """DMA + project + evict one chunk of K and V (xq streamed along)."""
            kt = xin.tile([P, ACH], bf16, tag="xin")
            nc.sync.dma_start(out=kt[:, :ch], in_=xk.ap()[:, j0 : j0 + ch])
            vt = xin.tile([P, ACH], bf16, tag="xin")
            nc.sync.dma_start(out=vt[:, :ch], in_=xv.ap()[:, j0 : j0 + ch])
            for m0 in range(0, ch, 2 * MM):
                ps = psA.tile([P, 2 * MM], f32, tag="psA")
                for u in (0, MM):
                    nc.tensor.matmul(
                        ps[:, u : u + MM], wk_s, kt[:, m0 + u : m0 + u + MM],
                        start=True, stop=True,
                    )
                nc.scalar.activation(
                    Kp[:, PAD * NG + j0 + m0 : PAD * NG + j0 + m0 + 2 * MM],
                    ps,
                    AF.Identity,
                    bias=bk_s,
                    scale=1.0,
                )
            return vt

        def emit_A_V(j0, ch, vt):
            for m0 in range(0, ch, 2 * MM):
                ps = psA.tile([P, 2 * MM], f32, tag="psA")
                for u in (0, MM):
                    nc.tensor.matmul(
                        ps[:, u : u + MM], wv_s, vt[:, m0 + u : m0 + u + MM],
                        start=True, stop=True,
                    )
                nc.scalar.activation(
                    Vp[:, PAD * NG + j0 + m0 : PAD * NG + j0 + m0 + 2 * MM],
                    ps,
                    AF.Identity,
                    bias=bv_s,
                    scale=1.0,
                )

        def emit_Q(base, fc):
            """Stream this chunk of xq in, project, evict (+bias) to bf16."""
            qx = work.tile([P, fc], bf16, tag="qx", bufs=3)
            nc.sync.dma_start(out=qx, in_=xq.ap()[:, base : base + fc])
            qb = work.tile([P, fc], bf16, tag="qb", bufs=3)
            qp = psA.tile([P, 2 * MM], f32, tag="psA")
            for u in (0, MM):
                nc.tensor.matmul(
                    qp[:, u : u + MM], wq_s, qx[:, u : u + MM], start=True, stop=True
                )
            nc.scalar.activation(qb, qp, AF.Identity, bias=bq_s, scale=1.0)
            return qb

        def emit_SE(base, fc, qb):
            """Batched score mul (all 7 windows in one op) + in-place exp."""
            seb = work.tile([P, KS, 1024], bf16, tag="seb", bufs=2)
            qb_b = bass.AP(tensor=qb.tensor, offset=qb.offset, ap=[qb.ap[0], [0, KS], [1, fc]])
            kp_v = bass.AP(
                tensor=Kp.tensor, offset=Kp.offset + base, ap=[Kp.ap[0], [NG, KS], [1, fc]]
            )
            nc.vector.tensor_mul(seb[:, :, :fc], qb_b, kp_v)
            nc.scalar.activation(seb[:, 0:4, :fc], seb[:, 0:4, :fc], AF.Exp)
            nc.scalar.activation(seb[:, 4:KS, :fc], seb[:, 4:KS, :fc], AF.Exp)
            return seb

        def emit_B_tail(base, fc, seb):
            """Value products, PSUM window accumulation, normalize, DMA out."""
            den = psB.tile([P, 1024], f32, tag="den", bufs=1)
            num = psB.tile([P, 1024], f32, tag="num", bufs=1)
            wb7 = work.tile([P, KS, 1024], bf16, tag="wb7", bufs=2)
            vp_a = bass.AP(
                tensor=Vp.tensor, offset=Vp.offset + base, ap=[Vp.ap[0], [NG, 4], [1, fc]]
            )
            vp_b = bass.AP(
                tensor=Vp.tensor,
                offset=Vp.offset + base + 4 * NG,
                ap=[Vp.ap[0], [NG, KS - 4], [1, fc]],
            )
            nc.vector.tensor_mul(wb7[:, 0:4, :fc], seb[:, 0:4, :fc], vp_a)
            nc.vector.tensor_mul(wb7[:, 4:KS, :fc], seb[:, 4:KS, :fc], vp_b)
            for i in range(KS):
                eb = seb[:, i, :fc]
                wb = wb7[:, i, :fc]
                first, last = i == 0, i == KS - 1
                for m0 in range(0, fc, MM):
                    nc.tensor.matmul(
                        den[:, m0 : m0 + MM],
                        id_s,
                        eb[:, m0 : m0 + MM],
                        start=first,
                        stop=last,
                        skip_group_check=True,
                    )
                    nc.tensor.matmul(
                        num[:, m0 : m0 + MM],
                        id_s,
                        wb[:, m0 : m0 + MM],
                        start=first,
                        stop=last,
                        skip_group_check=True,
                    )
            r = work.tile([P, fc], f32, tag="r", bufs=2)
            nc.vector.reciprocal_approx_fast(out=r, in_=den[:, :fc])
            ot = outp.tile([P, fc], bf16, tag="ot")
            nc.vector.tensor_mul(ot, num[:, :fc], r)
            nc.sync.dma_start(out=out_d.ap()[:, base : base + fc], in_=ot)

        # Interleave phase A and B with a two-deep software pipeline skew:
        # at iteration c we emit Q-projection for chunk c, score-mul+exp for
        # chunk c-1, and the value/accumulate/normalize block for chunk c-2.
        # This keeps the qb-evict -> score-mul -> exp chain off the critical
        # path between consecutive (large) exp ops.
        b_chunks = []
        base = 0
        for fc in CHUNKS:
            b_chunks.append((base, fc))
            base += fc
        ai = 0
        qbs = {}
        sebs = {}
        pending_v = []
        nb = len(b_chunks)
        for c in range(nb + 2):
            if c < nb:
                base, fc = b_chunks[c]
                need = min(F, base + fc + PAD * NG)  # interior coverage needed
                while ai < len(A_STEPS) and A_STEPS[ai][0] < need:
                    j0, ch = A_STEPS[ai]
                    pending_v.append((j0, ch, emit_A(j0, ch)))
                    ai += 1
                qbs[c] = emit_Q(base, fc)
            if c >= 1 and c - 1 < nb:
                base, fc = b_chunks[c - 1]
                sebs[c - 1] = emit_SE(base, fc, qbs.pop(c - 1))
            while pending_v:
                emit_A_V(*pending_v.pop(0))
            if c >= 2:
                base, fc = b_chunks[c - 2]
                emit_B_tail(base, fc, sebs.pop(c - 2))

    nc.compile()
    return nc


def _pack(x, bf):
    """[C, T, N] f32 -> [128, F] bf16: partition p = c + 64g, free = t*104 + n_loc."""
    xp = np.zeros((C, T, NPAD), np.float32)
    xp[:, :, :N] = x
    return np.ascontiguousarray(
        xp.reshape(C, T, 2, NG).transpose(2, 0, 1, 3).reshape(P, F)
    ).astype(bf)


def _unpack(o):
    """[128, F] -> [D, T, N]."""
    o = np.asarray(o, np.float32)
    return np.ascontiguousarray(
        o.reshape(2, D, T, NG).transpose(1, 2, 0, 3).reshape(D, T, NPAD)[:, :, :N]
    )


def _lhsT_blockdiag(W):
    Z = np.zeros((P, P), np.float32)
    Z[:C, :D] = W.T
    Z[C:, D:] = W.T
    return Z


def _prep_in_maps(q, k, v, Wq, bq, Wk, bk, Wv, bv):
    import ml_dtypes

    bf = ml_dtypes.bfloat16
    wts = np.concatenate(
        [
            _lhsT_blockdiag(np.asarray(Wq, np.float32)),
            _lhsT_blockdiag(np.asarray(Wk, np.float32)),
            _lhsT_blockdiag(np.asarray(Wv, np.float32)),
            np.eye(P, dtype=np.float32),
        ],
        axis=1,
    ).astype(bf)
    bia = np.stack(
        [np.concatenate([b, b]) for b in (bq, bk, bv)], axis=1
    ).astype(np.float32)
    in_maps = []
    for b in range(B):
        in_maps.append(
            {
                "xq": _pack(np.asarray(q[b], np.float32), bf),
                "xk": _pack(np.asarray(k[b], np.float32), bf),
                "xv": _pack(np.asarray(v[b], np.float32), bf),
                "wts": wts,
                "bia": bia,
            }
        )
    return in_maps


def run(inputs, trace=False):
    """Build (cached), run on 8 cores, return (output, BassKernelResults)."""
    from concourse.bass_utils import run_bass_kernel_spmd

    if "nc" not in _CACHE:
        _CACHE["nc"] = _build()
    nc = _CACHE["nc"]
    in_maps = _prep_in_maps(**inputs)
    res = run_bass_kernel_spmd(nc, in_maps, core_ids=list(range(B)), trace=trace)
    out = np.stack([_unpack(np.asarray(res.results[b]["out"])) for b in range(B)])
    return out, res


def kernel(q, k, v, Wq, bq, Wk, bk, Wv, bv):
    out, _ = run(dict(q=q, k=k, v=v, Wq=Wq, bq=bq, Wk=Wk, bk=bk, Wv=Wv, bv=bv))
    return out
